# revision 16
# baseline (speedup 1.0000x reference)
"""Dropless MoE FFN (router + top-2 dispatch + per-expert MLP + combine) on
8 Trainium2 NeuronCores.

Strategy (expert parallelism, per the sharding hint):
  - Router (softmax + top-2) runs on host in fp32 — it is ~0.02% of the
    FLOPs and IS the token dispatch: each of the 8 cores owns one expert
    and receives only the tokens routed to it (gather on host replaces the
    device all-to-all; full inputs in / full output out per the contract).
  - Each core computes y = (gelu_tanh(x_e @ w1_e) @ w2_e) * combine_weight
    for its ~1k routed tokens, entirely in bf16 matmuls (fp32 PSUM
    accumulation), weights read from HBM exactly once.
  - Host scatter-adds the two scaled expert outputs per token (combine).

Device kernel layout per core (C = token capacity = max routed count):
  GEMM1 (hT = w1.T-slices @ xT):  out[f_tile, tokens] so no transpose is
  ever needed between the two GEMMs; GELU applied PSUM->SBUF on ScalarE,
  producing bf16 hT resident in SBUF.  GEMM2 accumulates over all 32
  f-chunks into PSUM per 128-token tile (last tile ragged), scaled by the
  per-token combine weight during the PSUM->SBUF copy (ScalarE,
  per-partition scale).  Head DMAs are split across both HWDGE rings
  (sync + scalar) and interleaved (w1 chunk, xt chunk) so the PE starts
  within ~2us and ramps while the initial ~4MB streams in.
"""

import sys

for _p in ("/opt/trn_rl_repo",):
    if _p not in sys.path:
        sys.path.insert(0, _p)

import numpy as np
import ml_dtypes

BF16 = ml_dtypes.bfloat16

D_MODEL = 1024
D_FFN = 4096
N_EXPERTS = 8
TOP_K = 2
N_CORES = 8
P = 128                 # SBUF/PSUM partitions
KC = D_MODEL // P       # 8 contraction chunks for GEMM1
FC = D_FFN // P         # 32 f-chunks (contraction chunks for GEMM2)
MB = 4                  # w1 streamed in 4 blocks of 1024 f-columns

_kernel_cache: dict[int, object] = {}


def _token_groups(C):
    """Split C token columns into <=512-wide PSUM-bank-sized groups,
    as equal as possible (N>=256 keeps the PE issue overhead hidden)."""
    n_g = -(-C // 512)
    base, rem = divmod(C, n_g)
    sizes = [base + (1 if g < rem else 0) for g in range(n_g)]
    groups = []
    off = 0
    for sz in sizes:
        groups.append((off, sz))
        off += sz
    return groups


def _build(C):
    import concourse.bass as bass
    import concourse.mybir as mybir
    import concourse.tile as tile
    from concourse.tile_rust import add_dep_helper
    from concourse import bacc

    dt = mybir.dt
    AF = mybir.ActivationFunctionType
    TT = -(-C // P)                    # token tiles (last may be ragged)
    groups = _token_groups(C)

    nc = bacc.Bacc("TRN2", target_bir_lowering=False, debug=False,
                   num_devices=N_CORES)
    xt_d = nc.dram_tensor("xt", [KC, P, C], dt.bfloat16,
                          kind="ExternalInput").ap()
    w1_d = nc.dram_tensor("w1", [KC, P, D_FFN], dt.bfloat16,
                          kind="ExternalInput").ap()
    w2_d = nc.dram_tensor("w2", [FC // 4, P, 4, D_MODEL], dt.bfloat16,
                          kind="ExternalInput").ap()
    wt_d = nc.dram_tensor("wt", [P, TT], dt.float32,
                          kind="ExternalInput").ap()
    y_d = nc.dram_tensor("y", [TT, P, D_MODEL], dt.float32,
                         kind="ExternalOutput").ap()

    with tile.TileContext(nc) as tc:
        with (
            tc.tile_pool(name="xt", bufs=KC) as xt_pool,
            tc.tile_pool(name="w1", bufs=2 * KC) as w1_pool,
            tc.tile_pool(name="w2", bufs=FC // 4) as w2_pool,
            tc.tile_pool(name="ht", bufs=FC // 4) as ht_pool,
            tc.tile_pool(name="yo", bufs=2) as y_pool,
            tc.tile_pool(name="wt", bufs=1) as wt_pool,
            tc.tile_pool(name="ps1", bufs=6, space=bass.MemorySpace.PSUM) as ps1_pool,
            tc.tile_pool(name="ps2", bufs=2, space=bass.MemorySpace.PSUM) as ps2_pool,
        ):
            # w1 streamed in five f-column blocks; the small first block
            # (4 m-tiles, 1MB) shrinks the critical head bytes so the PE
            # starts sooner.  Later blocks are paced naturally by slot-WAR
            # (bufs = 2 blocks of 8 kc-tiles).
            BLOCKS = [(0, 4), (4, 4), (8, 8), (16, 8), (24, 8)]
            # Head: interleave (w1 block0 chunk -> sync ring, xt chunk ->
            # scalar ring) so GEMM1 m=0 can start as chunks land.
            w1_first = []
            xt_t = []
            for kc in range(KC):
                w = w1_pool.tile([P, BLOCKS[0][1] * P], dt.bfloat16,
                                 tag="w1", name=f"w1_0_{kc}")
                nc.sync.dma_start(w[:], w1_d[kc][:, :BLOCKS[0][1] * P])
                w1_first.append(w)
                t = xt_pool.tile([P, C], dt.bfloat16, tag="xt",
                                 name=f"xt_{kc}")
                nc.scalar.dma_start(t[:], xt_d[kc])
                xt_t.append(t)
            wt_t = wt_pool.tile([P, TT], dt.float32, tag="wt")
            nc.scalar.dma_start(wt_t[:], wt_d[:])

            # ---- GEMM1: hT[m*128+p, t] = sum_k w1[k, f] * x[t, k], + GELU
            ht_t = []
            gelu_insts = []
            w1_t = w1_first
            blk_of_m = {}
            for b, (m0, bm) in enumerate(BLOCKS):
                for m in range(m0, m0 + bm):
                    blk_of_m[m] = (b, m0, bm)
            ps_of_m = {}

            def emit_mms(m, kcs):
                b, m0, bm = blk_of_m[m]
                lhs_tiles = w1_of_b[b]
                for kc in kcs:
                    lhsT = lhs_tiles[kc][:, (m - m0) * P:(m - m0 + 1) * P]
                    for g, (off, sz) in enumerate(groups):
                        nc.tensor.matmul(ps_of_m[m][g][:, :sz], lhsT,
                                         xt_t[kc][:, off:off + sz],
                                         start=(kc == 0), stop=(kc == KC - 1))

            def emit_gelu(m):
                gelu_inst = None
                for g, (off, sz) in enumerate(groups):
                    gelu_inst = nc.scalar.activation(
                        ht_t[m // 4][:, m % 4, off:off + sz],
                        ps_of_m[m][g][:, :sz], AF.Gelu_apprx_tanh)
                gelu_insts.append(gelu_inst)
                del ps_of_m[m]

            w1_of_b = {0: w1_first}
            for m in range(FC):
                b, m0, bm = blk_of_m[m]
                if m == m0 and b > 0:
                    w1_t = [w1_pool.tile([P, bm * P], dt.bfloat16, tag="w1",
                                         name=f"w1_{b}_{kc}")
                            for kc in range(KC)]
                    for kc in range(KC):
                        dma = nc.sync.dma_start(
                            w1_t[kc][:], w1_d[kc][:, m0 * P:(m0 + bm) * P])
                        if b == 1:
                            # block1's slots are free from the start; hold its
                            # 1MB back until the head-critical loads are in
                            add_dep_helper(dma.ins, gelu_insts[0].ins,
                                           sync=True,
                                           reason="pace w1 block1 after head")
                    w1_of_b[b] = w1_t
                ps_of_m[m] = [ps1_pool.tile([P, 512], dt.float32, tag="ps1",
                                            name=f"ps1_{m}_{g}")
                              for g in range(len(groups))]
                if m % 4 == 0:
                    ht = ht_pool.tile([P, 4, C], dt.bfloat16, tag="ht",
                                      name=f"ht_{m // 4}")
                    ht_t.append(ht)
                if m == 0:
                    # During the head the last xt chunk lands ~7us after the
                    # first; emit kc 0-6 for m=0 AND m=1 before either kc=7
                    # so the PE has fill work while xt[7] streams in.
                    emit_mms(0, range(KC - 1))
                    continue
                if m == 1:
                    emit_mms(1, range(KC - 1))
                    emit_mms(0, [KC - 1])
                    emit_gelu(0)
                    emit_mms(1, [KC - 1])
                    emit_gelu(1)
                    continue
                emit_mms(m, range(KC))
                emit_gelu(m)

            # w2 prefetch on the SWDGE ring (gpsimd is otherwise idle), each
            # chunk paced behind a later GELU so the 8MB of w2 never steals
            # HBM bandwidth from the critical head loads (xt + w1 gate the
            # PE ramp); all chunks land a few iterations before GEMM2 needs
            # them
            w2_t = []
            for j in range(FC // 4):
                w2t = w2_pool.tile([P, 4, D_MODEL], dt.bfloat16, tag="w2",
                                   name=f"w2_{j}")
                w2_dma = nc.gpsimd.dma_start(w2t[:], w2_d[j])
                pace = 6 + (j * 23) // max(FC // 4 - 1, 1)
                add_dep_helper(w2_dma.ins, gelu_insts[pace].ins, sync=True,
                               reason="pace w2 prefetch behind GEMM1 progress")
                w2_t.append(w2t)

            # ---- GEMM2: y[t, d] = (sum_f h[t, f] * w2[f, d]) * wt[t]
            # h-outer so each D-half's PSUM finishes early and its ACT copy
            # + store overlap the other half's matmuls (shortens the exposed
            # tail after the very last matmul)
            for tt in range(TT):
                tp = min(P, C - tt * P)         # ragged last token tile
                y_t = y_pool.tile([P, D_MODEL], dt.float32, tag="yo")
                for h in range(2):
                    ps = ps2_pool.tile([P, 512], dt.float32, tag="ps2",
                                       name=f"ps2_{tt}_{h}")
                    for fc in range(FC):
                        nc.tensor.matmul(
                            ps[:tp, :],
                            ht_t[fc // 4][:, fc % 4, tt * P:tt * P + tp],
                            w2_t[fc // 4][:, fc % 4, h * 512:(h + 1) * 512],
                            start=(fc == 0), stop=(fc == FC - 1))
                    nc.scalar.activation(y_t[:tp, h * 512:(h + 1) * 512],
                                         ps[:tp, :], AF.Copy,
                                         scale=wt_t[:tp, tt:tt + 1])
                    # always store all 128 partitions: a <128-partition DMA
                    # collapses onto a single SDMA engine (~21GB/s) and would
                    # expose ~12us on the final tile; the extra rows are
                    # garbage the host drops
                    nc.sync.dma_start(y_d[tt, :, h * 512:(h + 1) * 512],
                                      y_t[:, h * 512:(h + 1) * 512])

    nc.compile()
    return nc


def _route(x, router_w):
    """Replicate the reference router math (jax on CPU, fp32)."""
    import jax
    import jax.numpy as jnp

    with jax.default_device(jax.devices("cpu")[0]):
        xt = jnp.asarray(np.asarray(x, np.float32)).reshape(-1, D_MODEL)
        logits = xt @ jnp.asarray(np.asarray(router_w, np.float32))
        probs = jax.nn.softmax(logits, axis=-1)
        top_p, top_i = jax.lax.top_k(probs, TOP_K)
    return np.asarray(top_p), np.asarray(top_i)


def _run(x, router_w, w1, w2, trace=False):
    from concourse import bass_utils

    x = np.asarray(x, np.float32)
    w1 = np.asarray(w1, np.float32)
    w2 = np.asarray(w2, np.float32)
    B, S, _ = x.shape
    T = B * S
    xt = x.reshape(T, D_MODEL)

    top_p, top_i = _route(x, router_w)

    idxs, wts = [], []
    maxn = 0
    for e in range(N_EXPERTS):
        hit = top_i == e                       # [T, K]
        sel = hit.any(axis=1)
        idx = np.nonzero(sel)[0]
        w = (top_p * hit).sum(axis=1)[sel]     # combine weight per routed token
        idxs.append(idx)
        wts.append(w.astype(np.float32))
        maxn = max(maxn, len(idx))

    C = max(maxn, 2 * P)
    nc = _kernel_cache.get(C)
    if nc is None:
        nc = _build(C)
        _kernel_cache[C] = nc
    TT = -(-C // P)

    in_maps = []
    for e in range(N_EXPERTS):
        n = len(idxs[e])
        xg = np.zeros((C, D_MODEL), np.float32)
        xg[:n] = xt[idxs[e]]
        xtb = np.ascontiguousarray(xg.T).astype(BF16).reshape(KC, P, C)
        w1b = np.ascontiguousarray(w1[e].astype(BF16).reshape(KC, P, D_FFN))
        w2b = np.ascontiguousarray(w2[e].astype(BF16)
                                   .reshape(FC // 4, 4, P, D_MODEL)
                                   .transpose(0, 2, 1, 3))
        wpad = np.zeros(TT * P, np.float32)
        wpad[:n] = wts[e]
        wtb = np.ascontiguousarray(wpad.reshape(TT, P).T)
        in_maps.append({"xt": xtb, "w1": w1b, "w2": w2b, "wt": wtb})

    res = bass_utils.run_bass_kernel_spmd(
        nc, in_maps, core_ids=list(range(N_CORES)), trace=trace)

    out = np.zeros((T, D_MODEL), np.float32)
    for e in range(N_EXPERTS):
        n = len(idxs[e])
        y = np.asarray(res.results[e]["y"], np.float32).reshape(TT * P,
                                                                D_MODEL)
        out[idxs[e]] += y[:n]
    return out.reshape(B, S, D_MODEL), res


def kernel(**inputs):
    out, _ = _run(inputs["x"], inputs["router_w"], inputs["w1"], inputs["w2"])
    return out


# revision 17
# speedup vs baseline: 1.1923x; 1.1923x over previous
"""Dropless MoE FFN (router + top-2 dispatch + per-expert MLP + combine) on
8 Trainium2 NeuronCores.

Strategy (expert parallelism, per the sharding hint):
  - Router (softmax + top-2) runs on host in fp32 — it is ~0.02% of the
    FLOPs and IS the token dispatch: each of the 8 cores owns one expert
    and receives only the tokens routed to it (gather on host replaces the
    device all-to-all; full inputs in / full output out per the contract).
  - Each core computes y = (gelu_tanh(x_e @ w1_e) @ w2_e) * combine_weight
    for its ~1k routed tokens, entirely in bf16 matmuls (fp32 PSUM
    accumulation), weights read from HBM exactly once.
  - Host scatter-adds the two scaled expert outputs per token (combine).

Device kernel layout per core (C = token capacity = max routed count):
  GEMM1 (hT = w1.T-slices @ xT):  out[f_tile, tokens] so no transpose is
  ever needed between the two GEMMs; GELU applied PSUM->SBUF on ScalarE,
  producing bf16 hT resident in SBUF.  GEMM2 accumulates over all 32
  f-chunks into PSUM per 128-token tile (last tile ragged), scaled by the
  per-token combine weight during the PSUM->SBUF copy (ScalarE,
  per-partition scale).  Head DMAs are split across both HWDGE rings
  (sync + scalar) and interleaved (w1 chunk, xt chunk) so the PE starts
  within ~2us and ramps while the initial ~4MB streams in.
"""

import sys

for _p in ("/opt/trn_rl_repo",):
    if _p not in sys.path:
        sys.path.insert(0, _p)

import numpy as np
import ml_dtypes

BF16 = ml_dtypes.bfloat16

D_MODEL = 1024
D_FFN = 4096
N_EXPERTS = 8
TOP_K = 2
N_CORES = 8
P = 128                 # SBUF/PSUM partitions
KC = D_MODEL // P       # 8 contraction chunks for GEMM1
FC = D_FFN // P         # 32 f-chunks (contraction chunks for GEMM2)
MB = 4                  # w1 streamed in 4 blocks of 1024 f-columns

_kernel_cache: dict[int, object] = {}


def _token_groups(C):
    """Split C token columns into <=512-wide PSUM-bank-sized groups,
    as equal as possible (N>=256 keeps the PE issue overhead hidden)."""
    n_g = -(-C // 512)
    base, rem = divmod(C, n_g)
    sizes = [base + (1 if g < rem else 0) for g in range(n_g)]
    groups = []
    off = 0
    for sz in sizes:
        groups.append((off, sz))
        off += sz
    return groups


def _build(C):
    import concourse.bass as bass
    import concourse.mybir as mybir
    import concourse.tile as tile
    from concourse.tile_rust import add_dep_helper
    from concourse import bacc

    dt = mybir.dt
    AF = mybir.ActivationFunctionType
    TT = -(-C // P)                    # token tiles (last may be ragged)
    groups = _token_groups(C)

    nc = bacc.Bacc("TRN2", target_bir_lowering=False, debug=False,
                   num_devices=N_CORES)
    xt_d = nc.dram_tensor("xt", [KC, P, C], dt.bfloat16,
                          kind="ExternalInput").ap()
    w1_d = nc.dram_tensor("w1", [KC, P, D_FFN], dt.bfloat16,
                          kind="ExternalInput").ap()
    w2_d = nc.dram_tensor("w2", [FC // 4, P, 4, D_MODEL], dt.bfloat16,
                          kind="ExternalInput").ap()
    wt_d = nc.dram_tensor("wt", [P, TT], dt.float32,
                          kind="ExternalInput").ap()
    y_d = nc.dram_tensor("y", [TT, P, D_MODEL], dt.float32,
                         kind="ExternalOutput").ap()

    with tile.TileContext(nc) as tc:
        with (
            tc.tile_pool(name="xt", bufs=KC) as xt_pool,
            tc.tile_pool(name="w1", bufs=2 * KC) as w1_pool,
            tc.tile_pool(name="w2", bufs=FC // 4) as w2_pool,
            tc.tile_pool(name="ht", bufs=FC // 4) as ht_pool,
            tc.tile_pool(name="yo", bufs=2) as y_pool,
            tc.tile_pool(name="wt", bufs=1) as wt_pool,
            tc.tile_pool(name="ps1", bufs=6, space=bass.MemorySpace.PSUM) as ps1_pool,
            tc.tile_pool(name="ps2", bufs=2, space=bass.MemorySpace.PSUM) as ps2_pool,
        ):
            # w1 streamed in five f-column blocks; the small first block
            # (4 m-tiles, 1MB) shrinks the critical head bytes so the PE
            # starts sooner.  Later blocks are paced naturally by slot-WAR
            # (bufs = 2 blocks of 8 kc-tiles).
            BLOCKS = [(0, 4), (4, 4), (8, 8), (16, 8), (24, 8)]
            # Head: interleave (w1 block0 chunk -> sync ring, xt chunk ->
            # scalar ring) so GEMM1 m=0 can start as chunks land.
            w1_first = []
            xt_t = []
            for kc in range(KC):
                w = w1_pool.tile([P, BLOCKS[0][1] * P], dt.bfloat16,
                                 tag="w1", name=f"w1_0_{kc}")
                nc.sync.dma_start(w[:], w1_d[kc][:, :BLOCKS[0][1] * P])
                w1_first.append(w)
                t = xt_pool.tile([P, C], dt.bfloat16, tag="xt",
                                 name=f"xt_{kc}")
                nc.scalar.dma_start(t[:], xt_d[kc])
                xt_t.append(t)
            wt_t = wt_pool.tile([P, TT], dt.float32, tag="wt")
            nc.scalar.dma_start(wt_t[:], wt_d[:])

            # ---- GEMM1: hT[m*128+p, t] = sum_k w1[k, f] * x[t, k], + GELU
            ht_t = []
            gelu_insts = []
            w1_t = w1_first
            blk_of_m = {}
            for b, (m0, bm) in enumerate(BLOCKS):
                for m in range(m0, m0 + bm):
                    blk_of_m[m] = (b, m0, bm)
            for m in range(FC):
                b, m0, bm = blk_of_m[m]
                mi = m - m0
                if mi == 0 and b > 0:
                    w1_t = [w1_pool.tile([P, bm * P], dt.bfloat16, tag="w1",
                                         name=f"w1_{b}_{kc}")
                            for kc in range(KC)]
                    for kc in range(KC):
                        dma = nc.sync.dma_start(
                            w1_t[kc][:], w1_d[kc][:, m0 * P:(m0 + bm) * P])
                        if b == 1:
                            # block1's slots are free from the start; hold its
                            # 1MB back until the head-critical loads are in
                            add_dep_helper(dma.ins, gelu_insts[0].ins,
                                           sync=True,
                                           reason="pace w1 block1 after head")
                ps = [ps1_pool.tile([P, 512], dt.float32, tag="ps1",
                                    name=f"ps1_{m}_{g}")
                      for g in range(len(groups))]
                for kc in range(KC):
                    lhsT = w1_t[kc][:, mi * P:(mi + 1) * P]
                    for g, (off, sz) in enumerate(groups):
                        nc.tensor.matmul(ps[g][:, :sz], lhsT,
                                         xt_t[kc][:, off:off + sz],
                                         start=(kc == 0), stop=(kc == KC - 1))
                if m % 4 == 0:
                    ht = ht_pool.tile([P, 4, C], dt.bfloat16, tag="ht",
                                      name=f"ht_{m // 4}")
                    ht_t.append(ht)
                gelu_inst = None
                for g, (off, sz) in enumerate(groups):
                    gelu_inst = nc.scalar.activation(ht[:, m % 4, off:off + sz],
                                                     ps[g][:, :sz],
                                                     AF.Gelu_apprx_tanh)
                gelu_insts.append(gelu_inst)

            # w2 prefetch on the SWDGE ring (gpsimd is otherwise idle), each
            # chunk paced behind a later GELU so the 8MB of w2 never steals
            # HBM bandwidth from the critical head loads (xt + w1 gate the
            # PE ramp); all chunks land a few iterations before GEMM2 needs
            # them
            w2_t = []
            for j in range(FC // 4):
                w2t = w2_pool.tile([P, 4, D_MODEL], dt.bfloat16, tag="w2",
                                   name=f"w2_{j}")
                w2_dma = nc.gpsimd.dma_start(w2t[:], w2_d[j])
                pace = 6 + (j * 23) // max(FC // 4 - 1, 1)
                add_dep_helper(w2_dma.ins, gelu_insts[pace].ins, sync=True,
                               reason="pace w2 prefetch behind GEMM1 progress")
                w2_t.append(w2t)

            # ---- GEMM2: y[t, d] = (sum_f h[t, f] * w2[f, d]) * wt[t]
            # h-outer so each D-half's PSUM finishes early and its ACT copy
            # + store overlap the other half's matmuls (shortens the exposed
            # tail after the very last matmul)
            for tt in range(TT):
                tp = min(P, C - tt * P)         # ragged last token tile
                y_t = y_pool.tile([P, D_MODEL], dt.float32, tag="yo")
                for h in range(2):
                    ps = ps2_pool.tile([P, 512], dt.float32, tag="ps2",
                                       name=f"ps2_{tt}_{h}")
                    for fc in range(FC):
                        nc.tensor.matmul(
                            ps[:tp, :],
                            ht_t[fc // 4][:, fc % 4, tt * P:tt * P + tp],
                            w2_t[fc // 4][:, fc % 4, h * 512:(h + 1) * 512],
                            start=(fc == 0), stop=(fc == FC - 1))
                    nc.scalar.activation(y_t[:tp, h * 512:(h + 1) * 512],
                                         ps[:tp, :], AF.Copy,
                                         scale=wt_t[:tp, tt:tt + 1])
                    # always store all 128 partitions: a <128-partition DMA
                    # collapses onto a single SDMA engine (~21GB/s) and would
                    # expose ~12us on the final tile; the extra rows are
                    # garbage the host drops
                    nc.sync.dma_start(y_d[tt, :, h * 512:(h + 1) * 512],
                                      y_t[:, h * 512:(h + 1) * 512])

    nc.compile()
    return nc


def _route(x, router_w):
    """Replicate the reference router math (jax on CPU, fp32)."""
    import jax
    import jax.numpy as jnp

    with jax.default_device(jax.devices("cpu")[0]):
        xt = jnp.asarray(np.asarray(x, np.float32)).reshape(-1, D_MODEL)
        logits = xt @ jnp.asarray(np.asarray(router_w, np.float32))
        probs = jax.nn.softmax(logits, axis=-1)
        top_p, top_i = jax.lax.top_k(probs, TOP_K)
    return np.asarray(top_p), np.asarray(top_i)


def _run(x, router_w, w1, w2, trace=False):
    from concourse import bass_utils

    x = np.asarray(x, np.float32)
    w1 = np.asarray(w1, np.float32)
    w2 = np.asarray(w2, np.float32)
    B, S, _ = x.shape
    T = B * S
    xt = x.reshape(T, D_MODEL)

    top_p, top_i = _route(x, router_w)

    idxs, wts = [], []
    maxn = 0
    for e in range(N_EXPERTS):
        hit = top_i == e                       # [T, K]
        sel = hit.any(axis=1)
        idx = np.nonzero(sel)[0]
        w = (top_p * hit).sum(axis=1)[sel]     # combine weight per routed token
        idxs.append(idx)
        wts.append(w.astype(np.float32))
        maxn = max(maxn, len(idx))

    C = max(maxn, 2 * P)
    nc = _kernel_cache.get(C)
    if nc is None:
        nc = _build(C)
        _kernel_cache[C] = nc
    TT = -(-C // P)

    in_maps = []
    for e in range(N_EXPERTS):
        n = len(idxs[e])
        xg = np.zeros((C, D_MODEL), np.float32)
        xg[:n] = xt[idxs[e]]
        xtb = np.ascontiguousarray(xg.T).astype(BF16).reshape(KC, P, C)
        w1b = np.ascontiguousarray(w1[e].astype(BF16).reshape(KC, P, D_FFN))
        w2b = np.ascontiguousarray(w2[e].astype(BF16)
                                   .reshape(FC // 4, 4, P, D_MODEL)
                                   .transpose(0, 2, 1, 3))
        wpad = np.zeros(TT * P, np.float32)
        wpad[:n] = wts[e]
        wtb = np.ascontiguousarray(wpad.reshape(TT, P).T)
        in_maps.append({"xt": xtb, "w1": w1b, "w2": w2b, "wt": wtb})

    res = bass_utils.run_bass_kernel_spmd(
        nc, in_maps, core_ids=list(range(N_CORES)), trace=trace)

    out = np.zeros((T, D_MODEL), np.float32)
    for e in range(N_EXPERTS):
        n = len(idxs[e])
        y = np.asarray(res.results[e]["y"], np.float32).reshape(TT * P,
                                                                D_MODEL)
        out[idxs[e]] += y[:n]
    return out.reshape(B, S, D_MODEL), res


def kernel(**inputs):
    out, _ = _run(inputs["x"], inputs["router_w"], inputs["w1"], inputs["w2"])
    return out
